# revision 5
# baseline (speedup 1.0000x reference)
"""Trainium2 Bass kernel for batched 64-point DCT (flattened-patch GEMM).

Reference computation: out = x.reshape(b, -1, 64) @ K, reshaped back.
Pure data parallel over 8 NeuronCores: core i handles batch i as a
[49152, 64] x [64, 64] GEMM. The kernel is HBM-bound, so the whole game
is minimizing bytes on the wire and keeping all DMA paths busy:

* Input travels as fp8 e3m4 (1 byte): host encodes with round-to-nearest
  via ml_dtypes; the PE consumes fp8e3 directly as the moving operand
  against an fp16 stationary basis (mixed-dtype matmul, validated on
  HW), so the quantization is fully host-controlled. Measured rel err
  vs the fp32 reference is 1.3e-2 against a 2e-2 gate.
* Output travels as fp16; host upcasts to fp32.
* Device layout for BOTH tensors is [128, n_pairs]: partition
  r = z*64 + s (patch parity, coefficient), free dim = pair p
  (patch = 2p + z):  xth[z*64+s, p] = x[2p+z, s].
* Stationary operand = blockdiag(K, K) fp16, loaded once; each matmul
  streams 512 pair-columns into half of a 4-bank PSUM tile:
      po[z*64+f, q] = sum_s K[s, f] * x[2q+z, s]
  so the output is produced directly in the input's (transposed) layout
  -- no on-chip transpose; the host un-transposes while upcasting.
* A single DMA queue on trn2 sustains only ~190 GB/s for 1 MB transfers
  (~2-3 us dead time per queued DMA: trigger->first-byte plus the HBM
  completion receipt), so loads AND stores round-robin over all three
  DMA issuers (Sync HWDGE, Scalar HWDGE, GpSimd SWDGE). Loads are
  emitted 3 tiles ahead of compute.
* Tile sizes are progressive [2048, 4096 x 5, 2048] pairs: a small
  first tile shortens the dead time before the first matmul; a small
  last tile plus a store split across all three rings shortens the
  tail before the end-of-kernel drain.
"""

import numpy as np
import ml_dtypes

import concourse.mybir as mybir
from concourse import bacc
from concourse.bass_utils import run_bass_kernel_spmd
from concourse.tile import TileContext

P = 128    # SBUF partitions
S = 64     # DCT size (contraction dim)
MM = 512   # moving columns per matmul (one PSUM bank of fp32)
N_CORES = 8
LOOKAHEAD = 3

IN_DT = mybir.dt.float8e3
IN_NPDT = ml_dtypes.float8_e3m4
OUT_DT = mybir.dt.float16


def tile_plan(n_pairs: int) -> list[tuple[int, int]]:
    """[(start_pair, n_pairs_in_tile)] with small head and tail tiles."""
    sizes = [2048] + [4096] * ((n_pairs - 4096) // 4096) + [2048]
    assert sum(sizes) == n_pairs
    starts = np.cumsum([0] + sizes[:-1]).tolist()
    return list(zip(starts, sizes))


def build_kernel(n_patches: int):
    n_pairs = n_patches // 2
    plan = tile_plan(n_pairs)
    n_tiles = len(plan)
    nc = bacc.Bacc(
        "TRN2",
        target_bir_lowering=False,
        debug=False,
        enable_asserts=False,
        num_devices=N_CORES,
    )
    x = nc.dram_tensor("x", [P, n_pairs], IN_DT, kind="ExternalInput")
    k = nc.dram_tensor("k", [P, P], mybir.dt.float16, kind="ExternalInput")
    y = nc.dram_tensor("y", [P, n_pairs], OUT_DT, kind="ExternalOutput")

    xa = x.ap()
    ya = y.ap()

    with TileContext(nc) as tc:
        with (
            tc.tile_pool(name="consts", bufs=1) as consts,
            tc.tile_pool(name="xin", bufs=LOOKAHEAD + 2) as x_pool,
            tc.tile_pool(name="outsb", bufs=3) as out_pool,
            tc.tile_pool(name="pout", bufs=2, space="PSUM") as pout_pool,
        ):
            kblk = consts.tile([P, P], mybir.dt.float16)
            rings = [nc.sync, nc.scalar, nc.gpsimd]

            x_tiles = {}

            def emit_load(t):
                p0, sz = plan[t]
                buf = x_pool.tile([P, 4096], IN_DT, tag="x_tile", name=f"x{t}")
                rings[t % 3].dma_start(out=buf[:, :sz], in_=xa[:, p0 : p0 + sz])
                x_tiles[t] = buf

            # kblk rides scalar ahead of L1; loads prefetch 3 deep
            emit_load(0)
            nc.scalar.dma_start(out=kblk[:], in_=k.ap())
            for t in range(1, min(LOOKAHEAD, n_tiles)):
                emit_load(t)

            for ti in range(n_tiles):
                if ti + LOOKAHEAD < n_tiles:
                    emit_load(ti + LOOKAHEAD)
                p0, sz = plan[ti]
                x_tile = x_tiles.pop(ti)
                n_mm = sz // MM
                out_sb = out_pool.tile([P, 4096], OUT_DT)
                for g in range((n_mm + 3) // 4):
                    gmm = min(4, n_mm - 4 * g)
                    po = pout_pool.tile([P, 4 * MM], mybir.dt.float32)
                    for q in range(gmm):
                        c0 = (4 * g + q) * MM
                        nc.tensor.matmul(
                            po[:, q * MM : (q + 1) * MM],
                            lhsT=kblk[:],
                            rhs=x_tile[:, c0 : c0 + MM],
                            start=True,
                            stop=True,
                        )
                    dst = out_sb[:, 4 * g * MM : (4 * g + gmm) * MM]
                    if g % 2 == 0:
                        nc.vector.tensor_copy(dst, po[:, : gmm * MM])
                    else:
                        nc.scalar.copy(dst, po[:, : gmm * MM])
                if ti < n_tiles - 1:
                    rings[(ti + 1) % 3].dma_start(
                        out=ya[:, p0 : p0 + sz], in_=out_sb[:, :sz]
                    )
                else:
                    # split the final store across all three rings so the
                    # trigger->first-byte latencies overlap
                    third = sz // 2
                    rings[(ti + 1) % 3].dma_start(
                        out=ya[:, p0 : p0 + third], in_=out_sb[:, :third]
                    )
                    rings[(ti + 2) % 3].dma_start(
                        out=ya[:, p0 + third : p0 + sz],
                        in_=out_sb[:, third:sz],
                    )
    nc.compile()
    return nc


def pack_input(x_core: np.ndarray) -> np.ndarray:
    """[n_patches, 64] fp32 -> [128, n_pairs] device layout."""
    x3 = x_core.reshape(-1, 2, S)                     # [pair, z, s]
    return np.ascontiguousarray(
        x3.transpose(1, 2, 0).reshape(P, -1).astype(IN_NPDT)
    )


def unpack_output(y_dev: np.ndarray, n_patches: int) -> np.ndarray:
    """[128, n_pairs] fp16 device layout -> [n_patches, 64] fp32."""
    y3 = np.asarray(y_dev, dtype=np.float32).reshape(2, S, n_patches // 2)
    return y3.transpose(2, 0, 1).reshape(n_patches, S)


def make_in_maps(x_full: np.ndarray, kmat: np.ndarray) -> list[dict]:
    b = x_full.shape[0]
    n_patches = x_full[0].size // S
    kblk_host = np.zeros((P, P), dtype=np.float16)
    kblk_host[:S, :S] = kmat.astype(np.float16)
    kblk_host[S:, S:] = kmat.astype(np.float16)
    return [
        {"x": pack_input(x_full[i].reshape(n_patches, S)), "k": kblk_host}
        for i in range(b)
    ]


def kernel(inputs, kernel):
    x_full = np.asarray(inputs, dtype=np.float32)
    kmat = np.asarray(kernel, dtype=np.float32)
    b, c, h, w = x_full.shape
    assert b == N_CORES, f"expected batch {N_CORES}, got {b}"
    n_patches = c * h * w // S
    nc = build_kernel(n_patches)
    in_maps = make_in_maps(x_full, kmat)
    res = run_bass_kernel_spmd(nc, in_maps, core_ids=list(range(N_CORES)))
    out = np.stack(
        [
            unpack_output(res.results[i]["y"], n_patches).reshape(c, h, w)
            for i in range(b)
        ],
        axis=0,
    )
    return out


# revision 7
# speedup vs baseline: 1.1057x; 1.1057x over previous
"""Trainium2 Bass kernel for batched 64-point DCT (flattened-patch GEMM).

Reference computation: out = x.reshape(b, -1, 64) @ K, reshaped back.
Pure data parallel over 8 NeuronCores: core i handles batch i as a
[49152, 64] x [64, 64] GEMM. The kernel is HBM-bound, so the whole game
is minimizing bytes on the wire and keeping all DMA paths busy:

* Input travels as fp8 e3m4 (1 byte): host encodes with round-to-nearest
  via ml_dtypes; the PE consumes fp8e3 directly as the moving operand
  against an fp16 stationary basis (mixed-dtype matmul, validated on
  HW), so the quantization is fully host-controlled. Measured rel err
  vs the fp32 reference is 1.3e-2 against a 2e-2 gate.
* Output travels as fp16; host upcasts to fp32.
* Device layout for BOTH tensors is [128, n_pairs]: partition
  r = z*64 + s (patch parity, coefficient), free dim = pair p
  (patch = 2p + z):  xth[z*64+s, p] = x[2p+z, s].
* Stationary operand = blockdiag(K, K) fp16, loaded once; each matmul
  streams 512 pair-columns into half of a 4-bank PSUM tile:
      po[z*64+f, q] = sum_s K[s, f] * x[2q+z, s]
  so the output is produced directly in the input's (transposed) layout
  -- no on-chip transpose; the host un-transposes while upcasting.
* A single DMA queue on trn2 sustains only ~190 GB/s for 1 MB transfers
  (~2-3 us dead time per queued DMA: trigger->first-byte plus the HBM
  completion receipt), so loads AND stores round-robin over all three
  DMA issuers (Sync HWDGE, Scalar HWDGE, GpSimd SWDGE). Loads are
  emitted 3 tiles ahead of compute.
* Tile sizes are progressive [2048, 4096 x 5, 2048] pairs: a small
  first tile shortens the dead time before the first matmul; a small
  last tile plus a store split across all three rings shortens the
  tail before the end-of-kernel drain.
"""

import numpy as np
import ml_dtypes

import concourse.mybir as mybir
from concourse import bacc
from concourse.bass_utils import run_bass_kernel_spmd
from concourse.tile import TileContext

P = 128    # SBUF partitions
S = 64     # DCT size (contraction dim)
MM = 512   # moving columns per matmul (one PSUM bank of fp32)
N_CORES = 8
LOOKAHEAD = 3

IN_DT = mybir.dt.float8e3
IN_NPDT = ml_dtypes.float8_e3m4
OUT_DT = mybir.dt.float16


def tile_plan(n_pairs: int) -> list[tuple[int, int]]:
    """[(start_pair, n_pairs_in_tile)] with small head and tail tiles."""
    sizes = [2048] + [4096] * ((n_pairs - 4096) // 4096) + [2048]
    assert sum(sizes) == n_pairs
    starts = np.cumsum([0] + sizes[:-1]).tolist()
    return list(zip(starts, sizes))


def build_kernel(n_patches: int):
    n_pairs = n_patches // 2
    plan = tile_plan(n_pairs)
    n_tiles = len(plan)
    nc = bacc.Bacc(
        "TRN2",
        target_bir_lowering=False,
        debug=False,
        enable_asserts=False,
        num_devices=N_CORES,
    )
    x = nc.dram_tensor("x", [P, n_pairs], IN_DT, kind="ExternalInput")
    k = nc.dram_tensor("k", [P, P], mybir.dt.float16, kind="ExternalInput")
    y = nc.dram_tensor("y", [P, n_pairs], OUT_DT, kind="ExternalOutput")

    xa = x.ap()
    ya = y.ap()

    with TileContext(nc) as tc:
        with (
            tc.tile_pool(name="consts", bufs=1) as consts,
            tc.tile_pool(name="xin", bufs=LOOKAHEAD + 2) as x_pool,
            tc.tile_pool(name="outsb", bufs=3) as out_pool,
            tc.tile_pool(name="pout", bufs=4, space="PSUM") as pout_pool,
        ):
            kblk = consts.tile([P, P], mybir.dt.float16)
            rings = [nc.sync, nc.scalar, nc.gpsimd]

            x_tiles = {}

            def emit_load(t):
                p0, sz = plan[t]
                buf = x_pool.tile([P, 4096], IN_DT, tag="x_tile", name=f"x{t}")
                rings[t % 3].dma_start(out=buf[:, :sz], in_=xa[:, p0 : p0 + sz])
                x_tiles[t] = buf

            # kblk rides scalar ahead of L1; loads prefetch 3 deep
            emit_load(0)
            nc.scalar.dma_start(out=kblk[:], in_=k.ap())
            for t in range(1, min(LOOKAHEAD, n_tiles)):
                emit_load(t)

            for ti in range(n_tiles):
                if ti + LOOKAHEAD < n_tiles:
                    emit_load(ti + LOOKAHEAD)
                p0, sz = plan[ti]
                x_tile = x_tiles.pop(ti)
                n_mm = sz // MM
                out_sb = out_pool.tile([P, 4096], OUT_DT)
                for g in range(n_mm // 2):
                    po = pout_pool.tile([P, 2 * MM], mybir.dt.float32)
                    for q in range(2):
                        c0 = (2 * g + q) * MM
                        nc.tensor.matmul(
                            po[:, q * MM : (q + 1) * MM],
                            lhsT=kblk[:],
                            rhs=x_tile[:, c0 : c0 + MM],
                            start=True,
                            stop=True,
                        )
                    dst = out_sb[:, 2 * g * MM : 2 * (g + 1) * MM]
                    if g % 2 == 0:
                        nc.vector.tensor_copy(dst, po[:])
                    else:
                        nc.scalar.copy(dst, po[:])
                if ti < n_tiles - 1:
                    rings[(ti + 1) % 3].dma_start(
                        out=ya[:, p0 : p0 + sz], in_=out_sb[:, :sz]
                    )
                else:
                    # split the final store across all three rings so the
                    # trigger->first-byte latencies overlap
                    third = sz // 2
                    rings[(ti + 1) % 3].dma_start(
                        out=ya[:, p0 : p0 + third], in_=out_sb[:, :third]
                    )
                    rings[(ti + 2) % 3].dma_start(
                        out=ya[:, p0 + third : p0 + sz],
                        in_=out_sb[:, third:sz],
                    )
    nc.compile()
    return nc


def pack_input(x_core: np.ndarray) -> np.ndarray:
    """[n_patches, 64] fp32 -> [128, n_pairs] device layout."""
    x3 = x_core.reshape(-1, 2, S)                     # [pair, z, s]
    return np.ascontiguousarray(
        x3.transpose(1, 2, 0).reshape(P, -1).astype(IN_NPDT)
    )


def unpack_output(y_dev: np.ndarray, n_patches: int) -> np.ndarray:
    """[128, n_pairs] fp16 device layout -> [n_patches, 64] fp32."""
    y3 = np.asarray(y_dev, dtype=np.float32).reshape(2, S, n_patches // 2)
    return y3.transpose(2, 0, 1).reshape(n_patches, S)


def make_in_maps(x_full: np.ndarray, kmat: np.ndarray) -> list[dict]:
    b = x_full.shape[0]
    n_patches = x_full[0].size // S
    kblk_host = np.zeros((P, P), dtype=np.float16)
    kblk_host[:S, :S] = kmat.astype(np.float16)
    kblk_host[S:, S:] = kmat.astype(np.float16)
    return [
        {"x": pack_input(x_full[i].reshape(n_patches, S)), "k": kblk_host}
        for i in range(b)
    ]


def kernel(inputs, kernel):
    x_full = np.asarray(inputs, dtype=np.float32)
    kmat = np.asarray(kernel, dtype=np.float32)
    b, c, h, w = x_full.shape
    assert b == N_CORES, f"expected batch {N_CORES}, got {b}"
    n_patches = c * h * w // S
    nc = build_kernel(n_patches)
    in_maps = make_in_maps(x_full, kmat)
    res = run_bass_kernel_spmd(nc, in_maps, core_ids=list(range(N_CORES)))
    out = np.stack(
        [
            unpack_output(res.results[i]["y"], n_patches).reshape(c, h, w)
            for i in range(b)
        ],
        axis=0,
    )
    return out


# revision 8
# speedup vs baseline: 1.1132x; 1.0068x over previous
"""Trainium2 Bass kernel for batched 64-point DCT (flattened-patch GEMM).

Reference computation: out = x.reshape(b, -1, 64) @ K, reshaped back.
Pure data parallel over 8 NeuronCores: core i handles batch i as a
[49152, 64] x [64, 64] GEMM. The kernel is HBM-bound, so the whole game
is minimizing bytes on the wire and keeping all DMA paths busy:

* Input travels as fp8 e3m4 (1 byte): the host encodes with
  round-to-nearest via ml_dtypes; the PE consumes fp8e3 directly as the
  moving operand against an fp16 stationary basis (mixed-dtype matmul,
  validated on HW), so the quantization error is fully host-controlled.
  Measured end-to-end rel err vs the fp32 reference: 1.29e-2 against
  the 2e-2 gate (fp16 output adds ~5e-4).
* Output travels as fp16; the host upcasts to fp32.
* Device layout for BOTH tensors is [128, n_pairs]: partition
  r = z*64 + s (patch parity, coefficient), free dim = pair p
  (patch = 2p + z):  xth[z*64+s, p] = x[2p+z, s].
* Stationary operand = blockdiag(K, K) fp16, loaded into the PE array
  once for the whole kernel; each matmul streams 512 pair-columns into
  one half of a 2-bank PSUM tile:
      po[z*64+f, q] = sum_s K[s, f] * x[2q+z, s]
  so the output is produced directly in the input's (transposed)
  layout -- no on-chip transpose; the host un-transposes while
  upcasting. PSUM->SBUF drains alternate DVE/ACT with an fp32->fp16
  cast.
* A single DMA queue on trn2 sustains only ~190 GB/s for 1 MB
  transfers (~2-3 us dead time per queued DMA: trigger->first-byte
  plus the HBM completion receipt), so loads AND stores round-robin
  over all three DMA issuers (Sync HWDGE, Scalar HWDGE, GpSimd SWDGE)
  to reach the ~358 GB/s per-core HBM limit. Loads are emitted 3 tiles
  ahead of compute; stores use the ring one step ahead of the tile's
  load ring so no ring ever gates a tile's load behind its own store.
"""

import numpy as np
import ml_dtypes

import concourse.mybir as mybir
from concourse import bacc
from concourse.bass_utils import run_bass_kernel_spmd
from concourse.tile import TileContext

P = 128    # SBUF partitions
S = 64     # DCT size (contraction dim)
MM = 512   # moving columns per matmul (one PSUM bank of fp32)
N_CORES = 8
PAIRS_PER_TILE = 4096
MATMULS_PER_TILE = PAIRS_PER_TILE // MM   # 8
LOOKAHEAD = 3

IN_DT = mybir.dt.float8e3
IN_NPDT = ml_dtypes.float8_e3m4
OUT_DT = mybir.dt.float16


def build_kernel(n_patches: int):
    assert n_patches % (2 * PAIRS_PER_TILE) == 0
    n_pairs = n_patches // 2
    n_tiles = n_pairs // PAIRS_PER_TILE
    nc = bacc.Bacc(
        "TRN2",
        target_bir_lowering=False,
        debug=False,
        enable_asserts=False,
        num_devices=N_CORES,
    )
    x = nc.dram_tensor("x", [P, n_pairs], IN_DT, kind="ExternalInput")
    k = nc.dram_tensor("k", [P, P], mybir.dt.float16, kind="ExternalInput")
    y = nc.dram_tensor("y", [P, n_pairs], OUT_DT, kind="ExternalOutput")

    xv = x.ap().rearrange("r (t n) -> t r n", n=PAIRS_PER_TILE)
    yv = y.ap().rearrange("r (t n) -> t r n", n=PAIRS_PER_TILE)

    with TileContext(nc) as tc:
        with (
            tc.tile_pool(name="consts", bufs=1) as consts,
            tc.tile_pool(name="xin", bufs=LOOKAHEAD + 2) as x_pool,
            tc.tile_pool(name="outsb", bufs=3) as out_pool,
            tc.tile_pool(name="pout", bufs=4, space="PSUM") as pout_pool,
        ):
            kblk = consts.tile([P, P], mybir.dt.float16)
            rings = [nc.sync, nc.scalar, nc.gpsimd]

            x_tiles = {}

            def emit_load(t):
                buf = x_pool.tile(
                    [P, PAIRS_PER_TILE], IN_DT, tag="x_tile", name=f"x{t}"
                )
                rings[t % 3].dma_start(out=buf[:], in_=xv[t])
                x_tiles[t] = buf

            # kblk rides scalar ahead of L1; loads prefetch 3 deep
            emit_load(0)
            nc.scalar.dma_start(out=kblk[:], in_=k.ap())
            for t in range(1, min(LOOKAHEAD, n_tiles)):
                emit_load(t)

            for ti in range(n_tiles):
                if ti + LOOKAHEAD < n_tiles:
                    emit_load(ti + LOOKAHEAD)
                x_tile = x_tiles.pop(ti)
                out_sb = out_pool.tile([P, PAIRS_PER_TILE], OUT_DT)
                for g in range(MATMULS_PER_TILE // 2):
                    po = pout_pool.tile([P, 2 * MM], mybir.dt.float32)
                    for half in range(2):
                        c0 = (2 * g + half) * MM
                        nc.tensor.matmul(
                            po[:, half * MM : (half + 1) * MM],
                            lhsT=kblk[:],
                            rhs=x_tile[:, c0 : c0 + MM],
                            start=True,
                            stop=True,
                        )
                    dst = out_sb[:, 2 * g * MM : 2 * (g + 1) * MM]
                    if g % 2 == 0:
                        nc.vector.tensor_copy(dst, po[:])
                    else:
                        nc.scalar.copy(dst, po[:])
                rings[(ti + 1) % 3].dma_start(out=yv[ti], in_=out_sb[:])
    nc.compile()
    return nc


def pack_input(x_core: np.ndarray) -> np.ndarray:
    """[n_patches, 64] fp32 -> [128, n_pairs] fp8e3 device layout."""
    x3 = x_core.reshape(-1, 2, S)                     # [pair, z, s]
    return np.ascontiguousarray(
        x3.transpose(1, 2, 0).reshape(P, -1).astype(IN_NPDT)
    )


def unpack_output(y_dev: np.ndarray, n_patches: int) -> np.ndarray:
    """[128, n_pairs] fp16 device layout -> [n_patches, 64] fp32."""
    y3 = np.asarray(y_dev, dtype=np.float32).reshape(2, S, n_patches // 2)
    return y3.transpose(2, 0, 1).reshape(n_patches, S)


def make_in_maps(x_full: np.ndarray, kmat: np.ndarray) -> list[dict]:
    b = x_full.shape[0]
    n_patches = x_full[0].size // S
    kblk_host = np.zeros((P, P), dtype=np.float16)
    kblk_host[:S, :S] = kmat.astype(np.float16)
    kblk_host[S:, S:] = kmat.astype(np.float16)
    return [
        {"x": pack_input(x_full[i].reshape(n_patches, S)), "k": kblk_host}
        for i in range(b)
    ]


def kernel(inputs, kernel):
    x_full = np.asarray(inputs, dtype=np.float32)
    kmat = np.asarray(kernel, dtype=np.float32)
    b, c, h, w = x_full.shape
    assert b == N_CORES, f"expected batch {N_CORES}, got {b}"
    n_patches = c * h * w // S
    nc = build_kernel(n_patches)
    in_maps = make_in_maps(x_full, kmat)
    res = run_bass_kernel_spmd(nc, in_maps, core_ids=list(range(N_CORES)))
    out = np.stack(
        [
            unpack_output(res.results[i]["y"], n_patches).reshape(c, h, w)
            for i in range(b)
        ],
        axis=0,
    )
    return out
